# revision 21
# baseline (speedup 1.0000x reference)
"""KoLeo loss kernel for Trainium2, 8 NeuronCores.

Strategy (data-parallel brute-force 1-NN over L2-normalized rows):
  - Each core gets a row-PERMUTED copy of x with its own 1024 rows first, so
    the self-match diagonal always falls in columns 0..1023 (core-invariant
    program, as required by SPMD).
  - On device: normalize rows in f32, scale by S=8, cast to fp8e4 (gpsimd),
    then transpose on-chip via XBAR SBUF->SBUF DMA of u16-packed fp8 pairs:
    btp[p, q, 2r+i] = xn8[r, 2*(q*128+p)+i]  (k-order permuted, consistently).
  - Own 1024 rows are unpacked (byte copies) into plane-major xtown
    [128, 6, 1024] with plane c=2q+i so DoubleRow weights get aligned APs.
  - dots slab: 3 fp8 DoubleRow matmuls (K=256 each) per [128,512] block;
    lhsT = xtown plane pairs, rhs = byte-interleaved view of btp.
    The self-match diagonal is masked by an extra matmul in the accumulation
    group (lhsT = -2*S^2*I, rhs = one-hot block).
  - Running elementwise max over 512-col chunks on DVE; final row-max m.
  - pdist for normalized vectors: dist = sqrt(2 - 2*m/S^2)  (the reference's
    +EPS inside the diff perturbs the scalar loss by ~1e-8 rel - negligible).
  - loss partial per core = sum(log(dist + EPS)); host combines:
    loss = -(sum partials) / 8192.

fp8e4 quantization of the normalized vectors changes the loss by ~1.5e-4
relative (measured against the f32 reference on the actual input
distribution), far inside the 2e-2 gate.
"""

import sys

sys.path.insert(0, "/opt/trn_rl_repo")

import numpy as np

import concourse.bass as bass
import concourse.mybir as mybir
import concourse.tile as tile
from concourse import bacc
from concourse.bass_utils import run_bass_kernel_spmd

B = 8192
D = 768
NCORES = 8
RPC = B // NCORES  # 1024 rows per core
P = 128
KC = D // P  # 6 contraction chunks
CH = 512  # moving chunk width
NCH = B // CH  # 16 chunks
T = RPC // P  # 8 row tiles per core
NJ = B // P  # 64 row tiles of the full x
JPL = 4  # row tiles per input DMA
EPS = 1e-8
S = 8.0  # fp8 prescale
NWARM = 72  # PE warm-up matmuls

f32 = mybir.dt.float32
f16 = mybir.dt.float16
f8 = mybir.dt.float8e4
u16 = mybir.dt.uint16
u8 = mybir.dt.uint8
AF = mybir.ActivationFunctionType
ALU = mybir.AluOpType
AX = mybir.AxisListType
PM = mybir.MatmulPerfMode


def _build_program():
    nc = bacc.Bacc("TRN2", target_bir_lowering=False, debug=False, enable_asserts=True)
    x_in = nc.dram_tensor("xm", [B, D], f32, kind="ExternalInput").ap()
    mdiag_in = nc.dram_tensor("mdiag", [P, P], f8, kind="ExternalInput").ap()
    monehot_in = nc.dram_tensor("monehot", [P, 4, CH], f8, kind="ExternalInput").ap()
    ones_in = nc.dram_tensor("ones", [P, 1], f32, kind="ExternalInput").ap()
    consts_in = nc.dram_tensor("consts", [P, 2], f32, kind="ExternalInput").ap()
    out_t = nc.dram_tensor("partial", [1, 1], f32, kind="ExternalOutput").ap()

    with tile.TileContext(nc) as tc:
        with (
            tc.tile_pool(name="big", bufs=1) as big,
            tc.tile_pool(name="ldp", bufs=4) as ldp,
            tc.tile_pool(name="work", bufs=2) as work,
            tc.tile_pool(name="xwork", bufs=4) as xwork,
            tc.tile_pool(name="small", bufs=4) as small,
            tc.tile_pool(name="pmm", bufs=6, space="PSUM") as pmm,
            tc.tile_pool(name="pwm", bufs=1, space="PSUM") as pwm,
            tc.tile_pool(name="pfi", bufs=1, space="PSUM") as pfi,
        ):
            # persistent tiles
            btp = big.tile([P, 3, 2 * B], f8, tag="btp")  # u16-packed xT
            xtown = big.tile([P, KC, RPC], f8, tag="xtown")  # own rows, planes
            mdiag = big.tile([P, P], f8, tag="mdiag")
            monehot = big.tile([P, 4, CH], f8, tag="monehot")
            ones = big.tile([P, 1], f32, tag="ones")
            consts = big.tile([P, 2], f32, tag="consts")
            accs = [big.tile([P, CH], f32, name=f"acc{t}", tag=f"acc{t}") for t in range(T)]
            rmax = big.tile([P, T], f32, tag="rmax")

            nc.sync.dma_start(mdiag[:], mdiag_in)
            nc.sync.dma_start(monehot[:], monehot_in)
            nc.sync.dma_start(ones[:], ones_in)
            nc.sync.dma_start(consts[:], consts_in)
            two = consts[:, 0:1]
            epsb = consts[:, 1:2]

            btp16 = btp[:].bitcast(u16)  # [128, 3, 8192]
            btpu8 = btp[:].bitcast(u8)

            # PE warm-up: matmuls on the constant tiles to lift the HAM clock
            # gate before the first real chunk is ready (~3.4us of activity)
            wpt = pwm.tile([P, CH], f32, tag="wpt")
            for w in range(NWARM):
                nc.tensor.matmul(
                    wpt[:],
                    lhsT=mdiag[:],
                    rhs=monehot[:, w % 4, :],
                    start=True,
                    stop=True,
                )

            def emit_chunk(n):
                # dots block rows: all 8 own tiles, cols: chunk n (512 wide)
                for t in range(T):
                    pt = pmm.tile([P, CH], f32, tag="pt")
                    diag = n == t // 4
                    for q in range(3):
                        nc.tensor.matmul(
                            pt[:],
                            lhsT=xtown[:, 2 * q : 2 * q + 2, t * P : (t + 1) * P],
                            rhs=btp[:, q, n * 2 * CH : (n + 1) * 2 * CH].rearrange(
                                "p (n2 i) -> p i n2", i=2
                            ),
                            start=(q == 0),
                            stop=(q == 2 and not diag),
                            perf_mode=PM.DoubleRow,
                        )
                    if diag:
                        # mask self-match: adds -2*S^2 at [p, (t%4)*128+p]
                        nc.tensor.matmul(
                            pt[:],
                            lhsT=mdiag[:],
                            rhs=monehot[:, t % 4, :],
                            start=False,
                            stop=True,
                        )
                    if n == 0:
                        nc.vector.tensor_copy(out=accs[t][:], in_=pt[:])
                    else:
                        nc.vector.tensor_tensor(
                            out=accs[t][:], in0=accs[t][:], in1=pt[:], op=ALU.max
                        )
                    if n == NCH - 1:
                        nc.vector.tensor_reduce(
                            rmax[:, t : t + 1], accs[t][:], axis=AX.X, op=ALU.max
                        )

            # Phase A pipeline: batched load -> normalize -> fp8 -> XBAR
            # transpose; dots chunks interleave as soon as columns complete.
            NG = NJ // JPL

            def dispatch_load(g):
                rt4 = ldp.tile([P, JPL, D], f32, tag="rt4")
                # input loads ride the Activation HWDGE ring so the sync
                # queue stays free for XBAR transpose dispatch
                nc.scalar.dma_start(
                    rt4[:],
                    x_in[g * JPL * P : (g + 1) * JPL * P, :].rearrange(
                        "(s p) d -> p s d", p=P
                    ),
                )
                return rt4

            # software-pipelined load dispatch: 2 groups of lookahead
            rt4s = {0: dispatch_load(0), 1: dispatch_load(1)}
            for jg in range(NG):
                if jg + 2 < NG:
                    rt4s[jg + 2] = dispatch_load(jg + 2)
                rt4 = rt4s.pop(jg)
                ss4 = small.tile([P, JPL], f32, tag="ss4")
                nrm4 = small.tile([P, JPL], f32, tag="nrm4")
                rinv4 = small.tile([P, JPL], f32, tag="rinv4")
                for s in range(JPL):
                    rt = rt4[:, s, :]
                    sq = work.tile([P, D], f32, tag="sq")
                    nc.scalar.activation(
                        sq[:], rt, AF.Square, accum_out=ss4[:, s : s + 1]
                    )
                # norm/S for the group (prescale folded into the sqrt)
                nc.scalar.activation(nrm4[:], ss4[:], AF.Sqrt, scale=1.0 / (S * S))
                nc.vector.reciprocal(rinv4[:], nrm4[:])
                for s in range(JPL):
                    j = jg * JPL + s
                    rt = rt4[:, s, :]
                    xn8 = xwork.tile([P, D], f8, tag="xn8")
                    # split the fp8 normalize-mul between DVE and ACT
                    if s % 2 == 0:
                        nc.vector.tensor_scalar_mul(xn8[:], rt, rinv4[:, s : s + 1])
                    else:
                        nc.scalar.mul(xn8[:], rt, rinv4[:, s : s + 1])
                    nc.sync.dma_start_transpose(
                        btp16[:, :, j * P : (j + 1) * P], xn8[:].bitcast(u16)
                    )
                    if j == T - 1:
                        # unpack own rows into plane-major layout:
                        # xtown[:, 2q+i, :] = btp bytes [:, q, i::2] (first 1024)
                        xtu8 = xtown[:].bitcast(u8)
                        for q in range(3):
                            src2 = btpu8[:, q, 0 : 2 * RPC].rearrange(
                                "p (r i) -> p i r", i=2
                            )
                            for i in range(2):
                                nc.vector.tensor_copy(
                                    out=xtu8[:, 2 * q + i, :], in_=src2[:, i, :]
                                )
                        emit_chunk(0)
                        emit_chunk(1)
                    elif j > T - 1 and j % 4 == 3:
                        emit_chunk(j // 4)

            # Phase C: dist -> log -> partial sum (rmax filled by the fused
            # tensor_tensor_reduce on the last chunk)
            dist = big.tile([P, T], f32, tag="dist")
            nc.scalar.activation(
                dist[:], rmax[:], AF.Sqrt, scale=-2.0 / (S * S), bias=two
            )
            logd = big.tile([P, T], f32, tag="logd")
            lsum = big.tile([P, 1], f32, tag="lsum")
            nc.scalar.activation(
                logd[:], dist[:], AF.Ln, bias=epsb, accum_out=lsum[:]
            )
            pfin = pfi.tile([1, 1], f32, tag="pfin")
            nc.tensor.matmul(pfin[:], lhsT=ones[:], rhs=lsum[:], start=True, stop=True)
            res = big.tile([1, 1], f32, tag="res")
            nc.vector.tensor_copy(out=res[:], in_=pfin[:])
            nc.sync.dma_start(out_t[:], res[:])

    nc.compile()
    return nc


_NC_CACHE = None


def _get_nc():
    global _NC_CACHE
    if _NC_CACHE is None:
        _NC_CACHE = _build_program()
    return _NC_CACHE


def _fp8(a: np.ndarray) -> np.ndarray:
    import ml_dtypes

    return a.astype(ml_dtypes.float8_e4m3).view(np.uint8)


def _make_in_maps(x: np.ndarray):
    mdiag = _fp8(-2.0 * S * S * np.eye(P, dtype=np.float32))
    monehot = np.zeros((P, 4, CH), dtype=np.float32)
    for v in range(4):
        monehot[:, v, v * P : (v + 1) * P] = np.eye(P, dtype=np.float32)
    monehot = _fp8(monehot)
    ones = np.ones((P, 1), dtype=np.float32)
    consts = np.zeros((P, 2), dtype=np.float32)
    consts[:, 0] = 2.0
    consts[:, 1] = EPS
    in_maps = []
    for m in range(NCORES):
        own = x[m * RPC : (m + 1) * RPC]
        rest = np.concatenate([x[: m * RPC], x[(m + 1) * RPC :]], axis=0)
        xm = np.ascontiguousarray(np.concatenate([own, rest], axis=0))
        in_maps.append(
            {
                "xm": xm,
                "mdiag": mdiag,
                "monehot": monehot,
                "ones": ones,
                "consts": consts,
            }
        )
    return in_maps


def kernel(student_output: np.ndarray) -> np.ndarray:
    x = np.asarray(student_output, dtype=np.float32)
    nc = _get_nc()
    in_maps = _make_in_maps(x)
    res = run_bass_kernel_spmd(nc, in_maps, list(range(NCORES)))
    total = 0.0
    for r in res.results:
        total += float(r["partial"].reshape(()))
    loss = -(total / B)
    return np.float32(loss)


# revision 24
# speedup vs baseline: 1.0073x; 1.0073x over previous
"""KoLeo loss kernel for Trainium2, 8 NeuronCores.

Strategy (data-parallel brute-force 1-NN over L2-normalized rows):
  - Each core gets a row-PERMUTED copy of x with its own 1024 rows first, so
    the self-match diagonal always falls in columns 0..1023 (core-invariant
    program, as required by SPMD).
  - On device: normalize rows in f32, scale by S=8, cast to fp8e4 (gpsimd),
    then transpose on-chip via XBAR SBUF->SBUF DMA of u16-packed fp8 pairs:
    btp[p, q, 2r+i] = xn8[r, 2*(q*128+p)+i]  (k-order permuted, consistently).
  - Own 1024 rows are unpacked (byte copies) into plane-major xtown
    [128, 6, 1024] with plane c=2q+i so DoubleRow weights get aligned APs.
  - dots slab: 3 fp8 DoubleRow matmuls (K=256 each) per [128,512] block;
    lhsT = xtown plane pairs, rhs = byte-interleaved view of btp.
    The self-match diagonal is masked by an extra matmul in the accumulation
    group (lhsT = -2*S^2*I, rhs = one-hot block).
  - Running elementwise max over 512-col chunks on DVE; final row-max m.
  - pdist for normalized vectors: dist = sqrt(2 - 2*m/S^2)  (the reference's
    +EPS inside the diff perturbs the scalar loss by ~1e-8 rel - negligible).
  - loss partial per core = sum(log(dist + EPS)); host combines:
    loss = -(sum partials) / 8192.

fp8e4 quantization of the normalized vectors changes the loss by ~1.5e-4
relative (measured against the f32 reference on the actual input
distribution), far inside the 2e-2 gate.
"""

import sys

sys.path.insert(0, "/opt/trn_rl_repo")

import numpy as np

import concourse.bass as bass
import concourse.mybir as mybir
import concourse.tile as tile
from concourse import bacc
from concourse.bass_utils import run_bass_kernel_spmd

B = 8192
D = 768
NCORES = 8
RPC = B // NCORES  # 1024 rows per core
P = 128
KC = D // P  # 6 contraction chunks
CH = 512  # moving chunk width
NCH = B // CH  # 16 chunks
T = RPC // P  # 8 row tiles per core
NJ = B // P  # 64 row tiles of the full x
JPL = 4  # row tiles per input DMA
EPS = 1e-8
S = 8.0  # fp8 prescale
NWARM = 72  # PE warm-up matmuls

f32 = mybir.dt.float32
f16 = mybir.dt.float16
f8 = mybir.dt.float8e4
u16 = mybir.dt.uint16
u8 = mybir.dt.uint8
AF = mybir.ActivationFunctionType
ALU = mybir.AluOpType
AX = mybir.AxisListType
PM = mybir.MatmulPerfMode


def _build_program():
    nc = bacc.Bacc("TRN2", target_bir_lowering=False, debug=False, enable_asserts=True)
    x_in = nc.dram_tensor("xm", [B, D], f32, kind="ExternalInput").ap()
    mdiag_in = nc.dram_tensor("mdiag", [P, P], f8, kind="ExternalInput").ap()
    monehot_in = nc.dram_tensor("monehot", [P, 4, CH], f8, kind="ExternalInput").ap()
    ones_in = nc.dram_tensor("ones", [P, 1], f32, kind="ExternalInput").ap()
    consts_in = nc.dram_tensor("consts", [P, 2], f32, kind="ExternalInput").ap()
    out_t = nc.dram_tensor("partial", [1, 1], f32, kind="ExternalOutput").ap()

    with tile.TileContext(nc) as tc:
        with (
            tc.tile_pool(name="big", bufs=1) as big,
            tc.tile_pool(name="ldp", bufs=4) as ldp,
            tc.tile_pool(name="work", bufs=2) as work,
            tc.tile_pool(name="xwork", bufs=10) as xwork,
            tc.tile_pool(name="small", bufs=4) as small,
            tc.tile_pool(name="pmm", bufs=6, space="PSUM") as pmm,
            tc.tile_pool(name="pwm", bufs=1, space="PSUM") as pwm,
            tc.tile_pool(name="pfi", bufs=1, space="PSUM") as pfi,
        ):
            # persistent tiles
            btp = big.tile([P, 3, 2 * B], f8, tag="btp")  # u16-packed xT
            xtown = big.tile([P, KC, RPC], f8, tag="xtown")  # own rows, planes
            mdiag = big.tile([P, P], f8, tag="mdiag")
            monehot = big.tile([P, 4, CH], f8, tag="monehot")
            ones = big.tile([P, 1], f32, tag="ones")
            consts = big.tile([P, 2], f32, tag="consts")
            accs = [big.tile([P, CH], f32, name=f"acc{t}", tag=f"acc{t}") for t in range(T)]
            rmax = big.tile([P, T], f32, tag="rmax")

            nc.sync.dma_start(mdiag[:], mdiag_in)
            nc.sync.dma_start(monehot[:], monehot_in)
            nc.sync.dma_start(ones[:], ones_in)
            nc.sync.dma_start(consts[:], consts_in)
            two = consts[:, 0:1]
            epsb = consts[:, 1:2]

            btp16 = btp[:].bitcast(u16)  # [128, 3, 8192]
            btpu8 = btp[:].bitcast(u8)

            # PE warm-up: matmuls on the constant tiles to lift the HAM clock
            # gate before the first real chunk is ready (~3.4us of activity)
            wpt = pwm.tile([P, CH], f32, tag="wpt")
            for w in range(NWARM):
                nc.tensor.matmul(
                    wpt[:],
                    lhsT=mdiag[:],
                    rhs=monehot[:, w % 4, :],
                    start=True,
                    stop=True,
                )

            def unpack_own(c0, c1):
                # own-rows columns [c0, c1): xtown[:, 2q+i, c] = fp8 byte i of
                # packed u16 col c in plane q
                xtu8 = xtown[:].bitcast(u8)
                for q in range(3):
                    src2 = btpu8[:, q, 2 * c0 : 2 * c1].rearrange(
                        "p (r i) -> p i r", i=2
                    )
                    for i in range(2):
                        nc.vector.tensor_copy(
                            out=xtu8[:, 2 * q + i, c0:c1], in_=src2[:, i, :]
                        )

            def emit_chunk(n, trange):
                # dots block rows: own tiles in trange, cols: chunk n
                for t in trange:
                    pt = pmm.tile([P, CH], f32, tag="pt")
                    diag = n == t // 4
                    for q in range(3):
                        nc.tensor.matmul(
                            pt[:],
                            lhsT=xtown[:, 2 * q : 2 * q + 2, t * P : (t + 1) * P],
                            rhs=btp[:, q, n * 2 * CH : (n + 1) * 2 * CH].rearrange(
                                "p (n2 i) -> p i n2", i=2
                            ),
                            start=(q == 0),
                            stop=(q == 2 and not diag),
                            perf_mode=PM.DoubleRow,
                        )
                    if diag:
                        # mask self-match: adds -2*S^2 at [p, (t%4)*128+p]
                        nc.tensor.matmul(
                            pt[:],
                            lhsT=mdiag[:],
                            rhs=monehot[:, t % 4, :],
                            start=False,
                            stop=True,
                        )
                    if n == 0:
                        nc.vector.tensor_copy(out=accs[t][:], in_=pt[:])
                    else:
                        nc.vector.tensor_tensor(
                            out=accs[t][:], in0=accs[t][:], in1=pt[:], op=ALU.max
                        )
                    if n == NCH - 1:
                        nc.vector.tensor_reduce(
                            rmax[:, t : t + 1], accs[t][:], axis=AX.X, op=ALU.max
                        )

            # Phase A pipeline: batched load -> normalize -> fp8 -> XBAR
            # transpose; dots chunks interleave as soon as columns complete.
            NG = NJ // JPL

            def dispatch_load(g):
                rt4 = ldp.tile([P, JPL, D], f32, tag="rt4")
                # input loads ride the Activation HWDGE ring so the sync
                # queue stays free for XBAR transpose dispatch
                nc.scalar.dma_start(
                    rt4[:],
                    x_in[g * JPL * P : (g + 1) * JPL * P, :].rearrange(
                        "(s p) d -> p s d", p=P
                    ),
                )
                return rt4

            # software-pipelined load dispatch: 2 groups of lookahead
            rt4s = {0: dispatch_load(0), 1: dispatch_load(1)}
            for jg in range(NG):
                if jg + 2 < NG:
                    rt4s[jg + 2] = dispatch_load(jg + 2)
                rt4 = rt4s.pop(jg)
                ss4 = small.tile([P, JPL], f32, tag="ss4")
                nrm4 = small.tile([P, JPL], f32, tag="nrm4")
                rinv4 = small.tile([P, JPL], f32, tag="rinv4")
                for s in range(JPL):
                    rt = rt4[:, s, :]
                    sq = work.tile([P, D], f32, tag="sq")
                    nc.scalar.activation(
                        sq[:], rt, AF.Square, accum_out=ss4[:, s : s + 1]
                    )
                # norm/S for the group (prescale folded into the sqrt)
                nc.scalar.activation(nrm4[:], ss4[:], AF.Sqrt, scale=1.0 / (S * S))
                nc.vector.reciprocal(rinv4[:], nrm4[:])
                for s in range(JPL):
                    j = jg * JPL + s
                    rt = rt4[:, s, :]
                    xn8 = xwork.tile([P, D], f8, tag="xn8")
                    # split the fp8 normalize-mul between DVE and ACT
                    if s % 2 == 0:
                        nc.vector.tensor_scalar_mul(xn8[:], rt, rinv4[:, s : s + 1])
                    else:
                        nc.scalar.mul(xn8[:], rt, rinv4[:, s : s + 1])
                    nc.sync.dma_start_transpose(
                        btp16[:, :, j * P : (j + 1) * P], xn8[:].bitcast(u16)
                    )
                    if j == 3:
                        # first half of own rows ready: unpack and start the
                        # first 4 row-tiles of chunk 0 early
                        unpack_own(0, RPC // 2)
                        emit_chunk(0, range(0, 4))
                    elif j == 7:
                        unpack_own(RPC // 2, RPC)
                        emit_chunk(0, range(4, 8))
                        emit_chunk(1, range(T))
                    elif j > T - 1 and j % 4 == 3:
                        emit_chunk(j // 4, range(T))

            # Phase C: dist -> log -> partial sum (rmax filled by the fused
            # tensor_tensor_reduce on the last chunk)
            dist = big.tile([P, T], f32, tag="dist")
            nc.scalar.activation(
                dist[:], rmax[:], AF.Sqrt, scale=-2.0 / (S * S), bias=two
            )
            logd = big.tile([P, T], f32, tag="logd")
            lsum = big.tile([P, 1], f32, tag="lsum")
            nc.scalar.activation(
                logd[:], dist[:], AF.Ln, bias=epsb, accum_out=lsum[:]
            )
            pfin = pfi.tile([1, 1], f32, tag="pfin")
            nc.tensor.matmul(pfin[:], lhsT=ones[:], rhs=lsum[:], start=True, stop=True)
            res = big.tile([1, 1], f32, tag="res")
            nc.vector.tensor_copy(out=res[:], in_=pfin[:])
            nc.sync.dma_start(out_t[:], res[:])

    nc.compile()
    return nc


_NC_CACHE = None


def _get_nc():
    global _NC_CACHE
    if _NC_CACHE is None:
        _NC_CACHE = _build_program()
    return _NC_CACHE


def _fp8(a: np.ndarray) -> np.ndarray:
    import ml_dtypes

    return a.astype(ml_dtypes.float8_e4m3).view(np.uint8)


def _make_in_maps(x: np.ndarray):
    mdiag = _fp8(-2.0 * S * S * np.eye(P, dtype=np.float32))
    monehot = np.zeros((P, 4, CH), dtype=np.float32)
    for v in range(4):
        monehot[:, v, v * P : (v + 1) * P] = np.eye(P, dtype=np.float32)
    monehot = _fp8(monehot)
    ones = np.ones((P, 1), dtype=np.float32)
    consts = np.zeros((P, 2), dtype=np.float32)
    consts[:, 0] = 2.0
    consts[:, 1] = EPS
    in_maps = []
    for m in range(NCORES):
        own = x[m * RPC : (m + 1) * RPC]
        rest = np.concatenate([x[: m * RPC], x[(m + 1) * RPC :]], axis=0)
        xm = np.ascontiguousarray(np.concatenate([own, rest], axis=0))
        in_maps.append(
            {
                "xm": xm,
                "mdiag": mdiag,
                "monehot": monehot,
                "ones": ones,
                "consts": consts,
            }
        )
    return in_maps


def kernel(student_output: np.ndarray) -> np.ndarray:
    x = np.asarray(student_output, dtype=np.float32)
    nc = _get_nc()
    in_maps = _make_in_maps(x)
    res = run_bass_kernel_spmd(nc, in_maps, list(range(NCORES)))
    total = 0.0
    for r in res.results:
        total += float(r["partial"].reshape(()))
    loss = -(total / B)
    return np.float32(loss)
